# revision 6
# baseline (speedup 1.0000x reference)
"""Bidirectional GRU encoder (Keras reset_after GRU, mask_zero) on 8 trn2 cores.

Problem: B=512, T=64, V=96, E=256, H=512.
  sequences [B,T,2H], enc_hid [B,2H], mask [B,T] = reference(src_ids, emb, W_f, U_f, b_f, W_b, U_b, b_b)

Sharding: 8 cores = 2 directions x 4 batch-quarters, fully data parallel
(no collectives). Every core runs the IDENTICAL Bass program: a forward
GRU over its [128, 64] id chunk; backward-direction cores receive
time-reversed ids and their outputs are re-reversed on the host.

Per-core algorithm:
  - x@W re-associated as a row gather: EW = emb@W + b_in (+ b_rec for the
    z,r gates) is a [96, 1536] table computed on device with the PE, spilled
    to a DRAM scratch, then x_proj rows are fetched per step with an
    indirect DMA keyed on the step's token ids.
  - recurrence h@U: 12 PE matmuls per step (stationary hT chunks [128,128],
    streaming U [128,512] slices) accumulating into 3 PSUM banks (z, r, h).
  - gates on ACT (Sigmoid only: tanh(x) = 2*sigmoid(2x)-1 so no activation
    table-set switching) and DVE; masked update folded in as
    h_new = h + m*sigmoid(-az) * (hh - h).
  - h kept in both layouts; PE transposes rebuild hT each step.

The TPB ISA has a single semaphore-wait slot per instruction and walrus
refuses a Matmult carrying two waits, so the kernel keeps every matmul at
<=1 new semaphore: each DMA'd matmul operand is packed into ONE tile (one
DMA, one semaphore) whose semaphore is absorbed into PE's clock by a tiny
self-referencing matmul in a fresh PSUM slot, and every PSUM->SBUF copy
runs on DVE so recurrent matmul dependencies merge into a single DVE wait.
"""

import sys

sys.path.insert(0, "/opt/trn_rl_repo")

import numpy as np

B, T, V, E, H = 512, 64, 96, 256, 512
G = 3 * H
P = 128
BL = 128  # batch rows per core (4 chunks x 2 directions = 8 cores)
KC = H // P  # 4 contraction chunks
NB = G // 512  # 3 psum banks (z, r, h)
EC = E // P  # 2 embedding contraction chunks

_CACHE = {}


def _build_nc():
    import concourse.bacc as bacc
    import concourse.bass as bass
    import concourse.tile as tile
    from concourse import mybir
    from concourse.masks import make_identity

    f32 = mybir.dt.float32
    i32 = mybir.dt.int32
    AF = mybir.ActivationFunctionType
    OP = mybir.AluOpType

    # Bacc (not Bass): its finalize() runs generate_event_semaphores /
    # move_matmul_waits_to_ldweights, which split multi-semaphore waits into
    # event-semaphore instructions -- walrus rejects >1 wait per instruction.
    nc = bacc.Bacc(None)

    ids_d = nc.declare_dram_parameter("ids", [BL, T], i32, isOutput=False)
    # packed weights: [128, chunks*cols] with contraction chunks side by side
    embT_d = nc.declare_dram_parameter("embT_p", [P, EC * V], f32, isOutput=False)
    W_d = nc.declare_dram_parameter("W_p", [P, EC * G], f32, isOutput=False)
    U_d = nc.declare_dram_parameter("U_p", [P, KC * G], f32, isOutput=False)
    # b_in + [b_rec_z, b_rec_r, 0], replicated to the V=96 EW rows
    bias_d = nc.declare_dram_parameter("bias_rep", [V, G], f32, isOutput=False)
    # b_rec_h replicated across the 128 batch rows (added to the h psum bank)
    brh_d = nc.declare_dram_parameter("brh_rep", [BL, H], f32, isOutput=False)
    seq_d = nc.declare_dram_parameter("seq", [BL, T, H], f32, isOutput=True)
    hid_d = nc.declare_dram_parameter("hid", [BL, H], f32, isOutput=True)

    with tile.TileContext(nc) as tc:
        with (
            tc.tile_pool(name="const", bufs=1) as cpool,
            tc.tile_pool(name="xp", bufs=4) as xpool,
            tc.tile_pool(name="gates", bufs=2) as gpool,
            tc.tile_pool(name="h", bufs=3) as hpool,
            tc.tile_pool(name="hT", bufs=8) as htpool,
            tc.tile_pool(name="psum", bufs=2, space="PSUM") as ppool,
            tc.tile_pool(name="dram", bufs=1, space="DRAM") as dpool,
        ):
            ident = cpool.tile([P, P], f32, tag="ident")
            make_identity(nc, ident[:])

            ids_sb = cpool.tile([BL, T], i32, tag="ids")
            nc.sync.dma_start(ids_sb[:], ids_d[:, :])
            maskf = cpool.tile([BL, T], f32, tag="maskf")
            nc.vector.tensor_scalar(
                out=maskf[:], in0=ids_sb[:], scalar1=0, scalar2=None,
                op0=OP.not_equal,
            )

            brh_sb = cpool.tile([BL, H], f32, tag="brh")
            nc.sync.dma_start(brh_sb[:], brh_d[:, :])

            U_sb = cpool.tile([P, KC * G], f32, tag="u")
            nc.sync.dma_start(U_sb[:], U_d[:, :])
            embT_sb = cpool.tile([P, EC * V], f32, tag="embT")
            nc.sync.dma_start(embT_sb[:], embT_d[:, :])
            W_sb = cpool.tile([P, EC * G], f32, tag="w")
            nc.sync.dma_start(W_sb[:], W_d[:, :])
            bias_sb = cpool.tile([V, G], f32, tag="bias")
            nc.sync.dma_start(bias_sb[:], bias_d[:, :])

            # semaphore absorbers (see module docstring); fresh PSUM slots:
            # rpz slot0, rpr slot0, rph slot0, rpz slot1.
            for ap, tag in ((U_sb, "rpz"), (embT_sb, "rpr"),
                            (W_sb, "rph"), (ident, "rpz")):
                scr = ppool.tile([1, 512], f32, tag=tag)
                nc.tensor.matmul(scr[:1, :1], lhsT=ap[:1, :1], rhs=ap[:1, :1],
                                 start=True, stop=True)

            # ---- EW = emb @ W + bias, built on the PE, spilled to DRAM ----
            EW_dram = dpool.tile([V, G], f32)
            for nb in range(NB):
                ps = ppool.tile([V, 512], f32, tag="rpz")
                for c in range(EC):
                    nc.tensor.matmul(
                        ps[:],
                        lhsT=embT_sb[:, c * V:(c + 1) * V],
                        rhs=W_sb[:, c * G + nb * 512:c * G + (nb + 1) * 512],
                        start=(c == 0), stop=(c == EC - 1),
                    )
                ew_st = gpool.tile([V, 512], f32, tag="ewst")
                nc.vector.tensor_add(
                    ew_st[:], ps[:], bias_sb[:, nb * 512:(nb + 1) * 512])
                nc.sync.dma_start(EW_dram[:, nb * 512:(nb + 1) * 512], ew_st[:])

            # ---- recurrence ----
            h_cur = hpool.tile([BL, H], f32)
            nc.vector.memset(h_cur[:], 0.0)
            hT_cur = []
            for kc in range(KC):
                ht = htpool.tile([P, BL], f32)
                nc.vector.memset(ht[:], 0.0)
                hT_cur.append(ht)

            for t in range(T):
                xp = xpool.tile([BL, G], f32)
                nc.gpsimd.indirect_dma_start(
                    out=xp[:], out_offset=None, in_=EW_dram[:, :],
                    in_offset=bass.IndirectOffsetOnAxis(ap=ids_sb[:, t:t + 1], axis=0),
                )

                ps_z = ppool.tile([BL, 512], f32, tag="rpz")
                ps_r = ppool.tile([BL, 512], f32, tag="rpr")
                ps_h = ppool.tile([BL, 512], f32, tag="rph")
                for nb, ps in enumerate((ps_z, ps_r, ps_h)):
                    for kc in range(KC):
                        nc.tensor.matmul(
                            ps[:], lhsT=hT_cur[kc][:],
                            rhs=U_sb[:, kc * G + nb * 512:kc * G + (nb + 1) * 512],
                            start=(kc == 0), stop=(kc == KC - 1),
                        )

                az = gpool.tile([BL, 512], f32, tag="az")
                nc.vector.tensor_add(az[:], xp[:, 0:512], ps_z[:])
                s = gpool.tile([BL, 512], f32, tag="s")
                nc.scalar.activation(s[:], az[:], AF.Sigmoid, scale=-1.0)  # 1-z
                ar = gpool.tile([BL, 512], f32, tag="ar")
                nc.vector.tensor_add(ar[:], xp[:, 512:1024], ps_r[:])
                r = gpool.tile([BL, 512], f32, tag="r")
                nc.scalar.activation(r[:], ar[:], AF.Sigmoid)
                rh = gpool.tile([BL, 512], f32, tag="rh")
                nc.vector.tensor_add(rh[:], ps_h[:], brh_sb[:])
                t3 = gpool.tile([BL, 512], f32, tag="t3")
                nc.vector.tensor_mul(t3[:], r[:], rh[:])
                ah = gpool.tile([BL, 512], f32, tag="ah")
                nc.vector.tensor_add(ah[:], t3[:], xp[:, 1024:1536])
                s2 = gpool.tile([BL, 512], f32, tag="s2")
                nc.scalar.activation(s2[:], ah[:], AF.Sigmoid, scale=2.0)
                hh = gpool.tile([BL, 512], f32, tag="hh")
                nc.vector.tensor_scalar(
                    out=hh[:], in0=s2[:], scalar1=2.0, scalar2=-1.0,
                    op0=OP.mult, op1=OP.add,
                )  # tanh(ah) = 2*sigmoid(2*ah) - 1
                d = gpool.tile([BL, 512], f32, tag="d")
                nc.vector.tensor_sub(d[:], hh[:], h_cur[:])
                e = gpool.tile([BL, 512], f32, tag="e")
                nc.vector.scalar_tensor_tensor(
                    out=e[:], in0=s[:], scalar=maskf[:, t:t + 1], in1=d[:],
                    op0=OP.mult, op1=OP.mult,
                )  # m * (1-z) * (hh - h)
                h_new = hpool.tile([BL, H], f32)
                nc.vector.tensor_add(h_new[:], h_cur[:], e[:])

                nc.sync.dma_start(seq_d[:, t, :], h_new[:])

                if t == T - 1:
                    h_cur = h_new
                    break
                pst = ppool.tile([P, 512], f32, tag="pst")
                hT_new = []
                for kc in range(KC):
                    nc.tensor.transpose(
                        pst[:, kc * P:(kc + 1) * P],
                        h_new[:, kc * P:(kc + 1) * P],
                        ident[:],
                    )
                    ht = htpool.tile([P, BL], f32)
                    nc.vector.tensor_copy(ht[:], pst[:, kc * P:(kc + 1) * P])
                    hT_new.append(ht)
                h_cur, hT_cur = h_new, hT_new

            nc.sync.dma_start(hid_d[:, :], h_cur[:])

    nc.finalize()
    return nc


def _get_nc():
    if "nc" not in _CACHE:
        _CACHE["nc"] = _build_nc()
    return _CACHE["nc"]


def _pack_rows(M, chunk):
    """[R, C] -> [chunk, (R//chunk)*C] with row-chunks side by side."""
    R, C = M.shape
    return np.concatenate([M[i * chunk:(i + 1) * chunk] for i in range(R // chunk)],
                          axis=1)


def _in_maps(src_ids, emb, W_f, U_f, b_f, W_b, U_b, b_b):
    f = np.float32
    maps = []
    for direction, (Wd, Ud, bd) in enumerate(((W_f, U_f, b_f), (W_b, U_b, b_b))):
        Wd = np.asarray(Wd, f)
        Ud = np.asarray(Ud, f)
        bd = np.asarray(bd, f)
        embT = np.asarray(emb, f).T  # [E, V]
        bias_row = bd[0].copy()
        bias_row[0:2 * H] += bd[1, 0:2 * H]  # fold b_rec for z,r gates
        shared = {
            "embT_p": np.ascontiguousarray(_pack_rows(embT, P)),
            "W_p": np.ascontiguousarray(_pack_rows(Wd, P)),
            "U_p": np.ascontiguousarray(_pack_rows(Ud, P)),
            "bias_rep": np.ascontiguousarray(np.tile(bias_row[None, :], (V, 1))),
            "brh_rep": np.ascontiguousarray(np.tile(bd[1, 2 * H:][None, :], (BL, 1))),
        }
        for q in range(4):
            ids = np.ascontiguousarray(src_ids[q * BL:(q + 1) * BL], dtype=np.int32)
            if direction == 1:
                ids = np.ascontiguousarray(ids[:, ::-1])
            maps.append({"ids": ids, **shared})
    return maps


def kernel(src_ids, emb, W_f, U_f, b_f, W_b, U_b, b_b):
    from concourse.bass_utils import run_bass_kernel_spmd

    nc = _get_nc()
    maps = _in_maps(src_ids, emb, W_f, U_f, b_f, W_b, U_b, b_b)
    res = run_bass_kernel_spmd(nc, maps, core_ids=list(range(8))).results

    seq_f = np.concatenate([res[q]["seq"] for q in range(4)], axis=0)
    seq_b = np.concatenate([res[4 + q]["seq"] for q in range(4)], axis=0)[:, ::-1, :]
    sequences = np.concatenate([seq_f, seq_b], axis=-1)
    hid_f = np.concatenate([res[q]["hid"] for q in range(4)], axis=0)
    hid_b = np.concatenate([res[4 + q]["hid"] for q in range(4)], axis=0)
    enc_hid = np.concatenate([hid_f, hid_b], axis=-1)
    mask = np.asarray(src_ids) != 0
    return sequences, enc_hid, mask


# revision 13
# speedup vs baseline: 1.6099x; 1.6099x over previous
"""Bidirectional GRU encoder (Keras reset_after GRU, mask_zero) on 8 trn2 cores.

Problem: B=512, T=64, V=96, E=256, H=512.
  sequences [B,T,2H], enc_hid [B,2H], mask [B,T] = reference(src_ids, emb, W_f, U_f, b_f, W_b, U_b, b_b)

Sharding: 8 cores = 2 directions x 4 batch-quarters, fully data parallel
(no collectives). Every core runs the IDENTICAL Bass program: a forward
GRU over its [128, 64] id chunk; backward-direction cores receive
time-reversed ids and their outputs are re-reversed on the host.

Per-core algorithm (per step t):
  - recurrence preactivation = [h | onehot(ids_t) | 1] @ [[U],[EW],[b]]:
    4 contraction chunks of hT (stationary, fp32r) streaming U, plus a
    97-row one-hot chunk streaming EW_zr = emb@W (+bias row) which folds
    the x-projection for the z and r gates directly into the same PSUM
    accumulation, plus a K=1 ones-row streaming b_rec_h into the h bank.
  - the h-gate x-projection (which must stay outside r*(.)) is fetched by
    indirect DMA from a [96, 512] DRAM table EW_h = emb@W_h + b_in_h.
  - gates: Sigmoid only (tanh(x) = 2*sigmoid(2x)-1, no ACT table switch):
      s  = sigmoid(-ps_z)            (= 1-z, ACT reads PSUM)
      r  = sigmoid(ps_r)
      t3 = r * ps_h;  ah = t3 + xph
      s2 = sigmoid(2*ah)
      c  = 2*s2 - h                  (fused DVE scalar_tensor_tensor)
      e1 = (c - 1) * s               (fused)
      h_new = e1 * m + h             (fused, m = per-partition mask col)
  - PE transposes (fp32r) rebuild the stationary hT layout each step.

All matmul operand tensors are float32r (fp32 storage, reduced-precision
multiply): fp32 matmul streams at 4 cycles/row on trn2, float32r at 1
(N>=256). Transposes and all DVE/ACT gate tensors stay plain float32.

Built on Bacc (not Bass): its finalize() runs generate_event_semaphores /
move_matmul_waits_to_ldweights, which split multi-semaphore waits into
event-semaphore instructions -- walrus rejects >1 wait per instruction.
"""

import sys

sys.path.insert(0, "/opt/trn_rl_repo")

import numpy as np

B, T, V, E, H = 512, 64, 96, 256, 512
G = 3 * H
P = 128
BL = 128  # batch rows per core (4 chunks x 2 directions = 8 cores)
KC = H // P  # 4 contraction chunks
EC = E // P  # 2 embedding contraction chunks

_CACHE = {}


def _build_nc():
    import concourse.bacc as bacc
    import concourse.bass as bass
    import concourse.tile as tile
    from concourse import mybir
    from concourse.masks import make_identity

    f32 = mybir.dt.float32
    f32r = mybir.dt.float32r
    i32 = mybir.dt.int32
    AF = mybir.ActivationFunctionType
    OP = mybir.AluOpType

    nc = bacc.Bacc(None)

    ids_d = nc.declare_dram_parameter("ids", [BL, T], i32, isOutput=False)
    idsT_d = nc.declare_dram_parameter("idsT", [T, BL], f32, isOutput=False)
    # packed weights: [128, chunks*cols] with contraction chunks side by side
    embT_d = nc.declare_dram_parameter("embT_p", [P, EC * V], f32r, isOutput=False)
    W_d = nc.declare_dram_parameter("W_p", [P, EC * G], f32r, isOutput=False)
    U_d = nc.declare_dram_parameter("U_p", [P, KC * G], f32r, isOutput=False)
    # b_in + b_rec for the z,r gate columns, one row [1, 1024]
    bzr_d = nc.declare_dram_parameter("bias_zr", [1, 2 * H], f32r, isOutput=False)
    # b_in for the h gate, replicated to the V=96 EW_h rows
    bh_d = nc.declare_dram_parameter("bias_h_rep", [V, H], f32, isOutput=False)
    # b_rec_h as a single row (streamed by a K=1 matmul into the h bank)
    brh_d = nc.declare_dram_parameter("brh_row", [1, H], f32r, isOutput=False)
    seq_d = nc.declare_dram_parameter("seq", [BL, T, H], f32, isOutput=True)
    hid_d = nc.declare_dram_parameter("hid", [BL, H], f32, isOutput=True)

    with tile.TileContext(nc) as tc:
        with (
            tc.tile_pool(name="const", bufs=1) as cpool,
            tc.tile_pool(name="xp", bufs=4) as xpool,
            tc.tile_pool(name="oh", bufs=4) as ohpool,
            tc.tile_pool(name="gates", bufs=2) as gpool,
            tc.tile_pool(name="h", bufs=3) as hpool,
            tc.tile_pool(name="hT", bufs=8) as htpool,
            tc.tile_pool(name="psum", bufs=2, space="PSUM") as ppool,
            tc.tile_pool(name="dram", bufs=1, space="DRAM") as dpool,
        ):
            ident = cpool.tile([P, P], f32, tag="ident")
            make_identity(nc, ident[:])

            ids_sb = cpool.tile([BL, T], i32, tag="ids")
            nc.sync.dma_start(ids_sb[:], ids_d[:, :])
            maskf = cpool.tile([BL, T], f32, tag="maskf")
            nc.vector.tensor_scalar(
                out=maskf[:], in0=ids_sb[:], scalar1=0, scalar2=None,
                op0=OP.not_equal,
            )
            iota_p = cpool.tile([P, 1], f32, tag="iota")
            nc.gpsimd.iota(iota_p[:], pattern=[[0, 1]], base=0, channel_multiplier=1,
                           allow_small_or_imprecise_dtypes=True)

            brh_sb = cpool.tile([1, H], f32r, tag="brh")
            nc.sync.dma_start(brh_sb[:], brh_d[:, :])
            ones_f = cpool.tile([1, BL], f32, tag="onesf")
            nc.vector.memset(ones_f[:], 1.0)
            ones_sb = cpool.tile([1, BL], f32r, tag="ones")
            nc.vector.tensor_copy(ones_sb[:], ones_f[:])

            U_sb = cpool.tile([P, KC * G], f32r, tag="u")
            nc.sync.dma_start(U_sb[:], U_d[:, :])
            embT_sb = cpool.tile([P, EC * V], f32r, tag="embT")
            nc.sync.dma_start(embT_sb[:], embT_d[:, :])
            W_sb = cpool.tile([P, EC * G], f32r, tag="w")
            nc.sync.dma_start(W_sb[:], W_d[:, :])
            bh_sb = cpool.tile([V, H], f32, tag="biasH")
            nc.sync.dma_start(bh_sb[:], bh_d[:, :])

            # semaphore absorbers: each DMA'd matmul operand's semaphore is
            # folded into PE's observed clock by a tiny self-referencing
            # matmul in a fresh PSUM slot, so real matmuls start wait-free.
            for ap, tag in ((U_sb, "rpz"), (embT_sb, "rpr"),
                            (W_sb, "rph"), (ident, "rpz")):
                scr = ppool.tile([1, 512], f32, tag=tag)
                a1 = ap[:1, :1].bitcast(f32)
                nc.tensor.matmul(scr[:1, :1], lhsT=a1, rhs=a1,
                                 start=True, stop=True)

            # ---- EW tables: emb@W, z/r half kept in SBUF (one-hot matmul
            # rhs, bias as row 96), h third spilled to DRAM for the gather.
            EWzr_sb = cpool.tile([P, 2 * H], f32r, tag="ewzr")
            nc.sync.dma_start(EWzr_sb[96:97, :], bzr_d[:, :])
            EWh_dram = dpool.tile([V, H], f32)
            for nb in range(3):
                ps = ppool.tile([V, 512], f32, tag="rpz")
                for c in range(EC):
                    nc.tensor.matmul(
                        ps[:],
                        lhsT=embT_sb[:, c * V:(c + 1) * V],
                        rhs=W_sb[:, c * G + nb * 512:c * G + (nb + 1) * 512],
                        start=(c == 0), stop=(c == EC - 1),
                    )
                if nb < 2:
                    nc.vector.tensor_copy(EWzr_sb[:V, nb * 512:(nb + 1) * 512], ps[:])
                else:
                    ew_st = gpool.tile([V, 512], f32, tag="ewst")
                    nc.vector.tensor_add(ew_st[:], ps[:], bh_sb[:])
                    nc.sync.dma_start(EWh_dram[:, :], ew_st[:])

            # ---- recurrence ----
            h_cur = hpool.tile([BL, H], f32)
            nc.vector.memset(h_cur[:], 0.0)
            zero_f = cpool.tile([P, BL], f32, tag="zerof")
            nc.vector.memset(zero_f[:], 0.0)
            hT_cur = []
            for kc in range(KC):
                ht = htpool.tile([P, BL], f32r)
                nc.vector.tensor_copy(ht[:], zero_f[:])
                hT_cur.append(ht)

            for t in range(T):
                # x-projection for the h gate: gather rows of EW_h
                xph = xpool.tile([BL, H], f32)
                nc.gpsimd.indirect_dma_start(
                    out=xph[:], out_offset=None, in_=EWh_dram[:, :],
                    in_offset=bass.IndirectOffsetOnAxis(ap=ids_sb[:, t:t + 1], axis=0),
                )

                # one-hot chunk [v, b] for this step: row v=id_b gets 1.0;
                # row 96 = 1.0 everywhere (bias row of EWzr)
                idsb = ohpool.tile([P, BL], f32, tag="idsb")
                nc.sync.dma_start(idsb[:], idsT_d[t:t + 1, :].to_broadcast([P, BL]))
                ohT = ohpool.tile([P, BL], f32r, tag="ohT")
                nc.gpsimd.tensor_scalar(
                    out=ohT[:], in0=idsb[:], scalar1=iota_p[:, :1], scalar2=None,
                    op0=OP.is_equal,
                )
                nc.gpsimd.tensor_copy(ohT[V:V + 1, :], ones_f[:, :])

                ps_z = ppool.tile([BL, 512], f32, tag="rpz")
                ps_r = ppool.tile([BL, 512], f32, tag="rpr")
                ps_h = ppool.tile([BL, 512], f32, tag="rph")
                for nb, ps in enumerate((ps_z, ps_r, ps_h)):
                    for kc in range(KC):
                        nc.tensor.matmul(
                            ps[:], lhsT=hT_cur[kc][:],
                            rhs=U_sb[:, kc * G + nb * 512:kc * G + (nb + 1) * 512],
                            start=(kc == 0), stop=False,
                        )
                # x-projection for z, r: one-hot row-gather fused into PSUM
                nc.tensor.matmul(
                    ps_z[:], lhsT=ohT[:V + 1, :], rhs=EWzr_sb[:V + 1, 0:512],
                    start=False, stop=True,
                )
                nc.tensor.matmul(
                    ps_r[:], lhsT=ohT[:V + 1, :], rhs=EWzr_sb[:V + 1, 512:1024],
                    start=False, stop=True,
                )
                # b_rec_h into the h bank
                nc.tensor.matmul(
                    ps_h[:], lhsT=ones_sb[:, :], rhs=brh_sb[:, :],
                    start=False, stop=True,
                )

                s = gpool.tile([BL, 512], f32, tag="s")
                nc.scalar.activation(s[:], ps_z[:], AF.Sigmoid, scale=-1.0)  # 1-z
                r = gpool.tile([BL, 512], f32, tag="r")
                nc.scalar.activation(r[:], ps_r[:], AF.Sigmoid)
                t3 = gpool.tile([BL, 512], f32, tag="t3")
                nc.vector.tensor_mul(t3[:], r[:], ps_h[:])
                ah = gpool.tile([BL, 512], f32, tag="ah")
                nc.vector.tensor_add(ah[:], t3[:], xph[:])
                s2 = gpool.tile([BL, 512], f32, tag="s2")
                nc.scalar.activation(s2[:], ah[:], AF.Sigmoid, scale=2.0)
                c = gpool.tile([BL, 512], f32, tag="c")
                nc.vector.scalar_tensor_tensor(
                    out=c[:], in0=s2[:], scalar=2.0, in1=h_cur[:],
                    op0=OP.mult, op1=OP.subtract,
                )  # 2*sigmoid(2*ah) - h = tanh(ah) - h + 1
                e1 = gpool.tile([BL, 512], f32, tag="e1")
                nc.vector.scalar_tensor_tensor(
                    out=e1[:], in0=c[:], scalar=-1.0, in1=s[:],
                    op0=OP.add, op1=OP.mult,
                )  # (hh - h) * (1-z)
                h_new = hpool.tile([BL, H], f32)
                nc.vector.scalar_tensor_tensor(
                    out=h_new[:], in0=e1[:], scalar=maskf[:, t:t + 1], in1=h_cur[:],
                    op0=OP.mult, op1=OP.add,
                )  # h + m*(1-z)*(hh-h)

                nc.sync.dma_start(seq_d[:, t, :], h_new[:])

                if t == T - 1:
                    h_cur = h_new
                    break
                pst = ppool.tile([P, 512], f32, tag="pst")
                hT_new = []
                for kc in range(KC):
                    nc.tensor.transpose(
                        pst[:, kc * P:(kc + 1) * P],
                        h_new[:, kc * P:(kc + 1) * P],
                        ident[:],
                    )
                    ht = htpool.tile([P, BL], f32r)
                    nc.scalar.copy(ht[:], pst[:, kc * P:(kc + 1) * P])
                    hT_new.append(ht)
                h_cur, hT_cur = h_new, hT_new

            nc.sync.dma_start(hid_d[:, :], h_cur[:])

    nc.finalize()
    return nc


def _get_nc():
    if "nc" not in _CACHE:
        _CACHE["nc"] = _build_nc()
    return _CACHE["nc"]


def _pack_rows(M, chunk):
    """[R, C] -> [chunk, (R//chunk)*C] with row-chunks side by side."""
    R, C = M.shape
    return np.concatenate([M[i * chunk:(i + 1) * chunk] for i in range(R // chunk)],
                          axis=1)


def _in_maps(src_ids, emb, W_f, U_f, b_f, W_b, U_b, b_b):
    f = np.float32
    maps = []
    for direction, (Wd, Ud, bd) in enumerate(((W_f, U_f, b_f), (W_b, U_b, b_b))):
        Wd = np.asarray(Wd, f)
        Ud = np.asarray(Ud, f)
        bd = np.asarray(bd, f)
        embT = np.asarray(emb, f).T  # [E, V]
        bias_zr = (bd[0, :2 * H] + bd[1, :2 * H])[None, :]
        shared = {
            "embT_p": np.ascontiguousarray(_pack_rows(embT, P)),
            "W_p": np.ascontiguousarray(_pack_rows(Wd, P)),
            "U_p": np.ascontiguousarray(_pack_rows(Ud, P)),
            "bias_zr": np.ascontiguousarray(bias_zr),
            "bias_h_rep": np.ascontiguousarray(
                np.tile(bd[0, 2 * H:][None, :], (V, 1))),
            "brh_row": np.ascontiguousarray(bd[1, 2 * H:][None, :]),
        }
        for q in range(4):
            ids = np.ascontiguousarray(src_ids[q * BL:(q + 1) * BL], dtype=np.int32)
            if direction == 1:
                ids = np.ascontiguousarray(ids[:, ::-1])
            maps.append({
                "ids": ids,
                "idsT": np.ascontiguousarray(ids.T.astype(np.float32)),
                **shared,
            })
    return maps


def kernel(src_ids, emb, W_f, U_f, b_f, W_b, U_b, b_b):
    from concourse.bass_utils import run_bass_kernel_spmd

    nc = _get_nc()
    maps = _in_maps(src_ids, emb, W_f, U_f, b_f, W_b, U_b, b_b)
    res = run_bass_kernel_spmd(nc, maps, core_ids=list(range(8))).results

    seq_f = np.concatenate([res[q]["seq"] for q in range(4)], axis=0)
    seq_b = np.concatenate([res[4 + q]["seq"] for q in range(4)], axis=0)[:, ::-1, :]
    sequences = np.concatenate([seq_f, seq_b], axis=-1)
    hid_f = np.concatenate([res[q]["hid"] for q in range(4)], axis=0)
    hid_b = np.concatenate([res[4 + q]["hid"] for q in range(4)], axis=0)
    enc_hid = np.concatenate([hid_f, hid_b], axis=-1)
    mask = np.asarray(src_ids) != 0
    return sequences, enc_hid, mask


# revision 14
# speedup vs baseline: 1.8872x; 1.1722x over previous
"""Bidirectional GRU encoder (Keras reset_after GRU, mask_zero) on 8 trn2 cores.

Problem: B=512, T=64, V=96, E=256, H=512.
  sequences [B,T,2H], enc_hid [B,2H], mask [B,T] = reference(src_ids, emb, W_f, U_f, b_f, W_b, U_b, b_b)

Sharding: 8 cores = 2 directions x 4 batch-quarters, fully data parallel
(no collectives). Every core runs the IDENTICAL Bass program: a forward
GRU over its [128, 64] id chunk; backward-direction cores receive
time-reversed ids and their outputs are re-reversed on the host.

Per-core algorithm (per step t):
  - recurrence preactivation = [h | onehot(ids_t) | 1] @ [[U],[EW],[b]]:
    4 contraction chunks of hT (stationary, fp32r) streaming U, plus a
    97-row one-hot chunk streaming EW_zr = emb@W (+bias row) which folds
    the x-projection for the z and r gates directly into the same PSUM
    accumulation, plus a K=1 ones-row streaming b_rec_h into the h bank.
  - the h-gate x-projection (which must stay outside r*(.)) is fetched by
    indirect DMA from a [96, 512] DRAM table EW_h = emb@W_h + b_in_h.
  - gates: Sigmoid only (tanh(x) = 2*sigmoid(2x)-1, no ACT table switch):
      s  = sigmoid(-ps_z)            (= 1-z, ACT reads PSUM)
      r  = sigmoid(ps_r)
      t3 = r * ps_h;  ah = t3 + xph
      s2 = sigmoid(2*ah)
      c  = 2*s2 - h                  (fused DVE scalar_tensor_tensor)
      e1 = (c - 1) * s               (fused)
      h_new = e1 * m + h             (fused, m = per-partition mask col)
  - PE transposes (fp32r) rebuild the stationary hT layout each step.

All matmul operand tensors are float32r (fp32 storage, reduced-precision
multiply): fp32 matmul streams at 4 cycles/row on trn2, float32r at 1
(N>=256). Transposes and all DVE/ACT gate tensors stay plain float32.

Built on Bacc (not Bass): its finalize() runs generate_event_semaphores /
move_matmul_waits_to_ldweights, which split multi-semaphore waits into
event-semaphore instructions -- walrus rejects >1 wait per instruction.
"""

import sys

sys.path.insert(0, "/opt/trn_rl_repo")

import numpy as np

B, T, V, E, H = 512, 64, 96, 256, 512
G = 3 * H
P = 128
BL = 128  # batch rows per core (4 chunks x 2 directions = 8 cores)
KC = H // P  # 4 contraction chunks
EC = E // P  # 2 embedding contraction chunks

_CACHE = {}


def _build_nc():
    import concourse.bacc as bacc
    import concourse.bass as bass
    import concourse.tile as tile
    from concourse import mybir
    from concourse.masks import make_identity

    f32 = mybir.dt.float32
    f32r = mybir.dt.float32r
    i32 = mybir.dt.int32
    AF = mybir.ActivationFunctionType
    OP = mybir.AluOpType

    nc = bacc.Bacc(None)

    ids_d = nc.declare_dram_parameter("ids", [BL, T], i32, isOutput=False)
    idsT_d = nc.declare_dram_parameter("idsT", [T, BL], f32, isOutput=False)
    # packed weights: [128, chunks*cols] with contraction chunks side by side
    embT_d = nc.declare_dram_parameter("embT_p", [P, EC * V], f32r, isOutput=False)
    W_d = nc.declare_dram_parameter("W_p", [P, EC * G], f32r, isOutput=False)
    U_d = nc.declare_dram_parameter("U_p", [P, KC * G], f32r, isOutput=False)
    # b_in + b_rec for the z,r gate columns, one row [1, 1024]
    bzr_d = nc.declare_dram_parameter("bias_zr", [1, 2 * H], f32r, isOutput=False)
    # b_in for the h gate, replicated to the V=96 EW_h rows
    bh_d = nc.declare_dram_parameter("bias_h_rep", [V, H], f32, isOutput=False)
    # b_rec_h as a single row (streamed by a K=1 matmul into the h bank)
    brh_d = nc.declare_dram_parameter("brh_row", [1, H], f32r, isOutput=False)
    seq_d = nc.declare_dram_parameter("seq", [BL, T, H], f32, isOutput=True)
    hid_d = nc.declare_dram_parameter("hid", [BL, H], f32, isOutput=True)

    with tile.TileContext(nc) as tc:
        with (
            tc.tile_pool(name="const", bufs=1) as cpool,
            tc.tile_pool(name="xp", bufs=4) as xpool,
            tc.tile_pool(name="oh", bufs=4) as ohpool,
            tc.tile_pool(name="gates", bufs=2) as gpool,
            tc.tile_pool(name="h", bufs=9) as hpool,
            tc.tile_pool(name="hT", bufs=8) as htpool,
            tc.tile_pool(name="psum", bufs=2, space="PSUM") as ppool,
            tc.tile_pool(name="dram", bufs=1, space="DRAM") as dpool,
        ):
            ident = cpool.tile([P, P], f32, tag="ident")
            make_identity(nc, ident[:])

            ids_sb = cpool.tile([BL, T], i32, tag="ids")
            nc.sync.dma_start(ids_sb[:], ids_d[:, :])
            maskf = cpool.tile([BL, T], f32, tag="maskf")
            nc.vector.tensor_scalar(
                out=maskf[:], in0=ids_sb[:], scalar1=0, scalar2=None,
                op0=OP.not_equal,
            )
            iota_p = cpool.tile([P, 1], f32, tag="iota")
            nc.gpsimd.iota(iota_p[:], pattern=[[0, 1]], base=0, channel_multiplier=1,
                           allow_small_or_imprecise_dtypes=True)

            brh_sb = cpool.tile([1, H], f32r, tag="brh")
            nc.sync.dma_start(brh_sb[:], brh_d[:, :])
            ones_f = cpool.tile([1, BL], f32, tag="onesf")
            nc.vector.memset(ones_f[:], 1.0)
            ones_sb = cpool.tile([1, BL], f32r, tag="ones")
            nc.vector.tensor_copy(ones_sb[:], ones_f[:])

            U_sb = cpool.tile([P, KC * G], f32r, tag="u")
            nc.sync.dma_start(U_sb[:], U_d[:, :])
            embT_sb = cpool.tile([P, EC * V], f32r, tag="embT")
            nc.sync.dma_start(embT_sb[:], embT_d[:, :])
            W_sb = cpool.tile([P, EC * G], f32r, tag="w")
            nc.sync.dma_start(W_sb[:], W_d[:, :])
            bh_sb = cpool.tile([V, H], f32, tag="biasH")
            nc.sync.dma_start(bh_sb[:], bh_d[:, :])

            # semaphore absorbers: each DMA'd matmul operand's semaphore is
            # folded into PE's observed clock by a tiny self-referencing
            # matmul in a fresh PSUM slot, so real matmuls start wait-free.
            for ap, tag in ((U_sb, "rpz"), (embT_sb, "rpr"),
                            (W_sb, "rph"), (ident, "rpz")):
                scr = ppool.tile([1, 512], f32, tag=tag)
                a1 = ap[:1, :1].bitcast(f32)
                nc.tensor.matmul(scr[:1, :1], lhsT=a1, rhs=a1,
                                 start=True, stop=True)

            # ---- EW tables: emb@W, z/r half kept in SBUF (one-hot matmul
            # rhs, bias as row 96), h third spilled to DRAM for the gather.
            EWzr_sb = cpool.tile([P, 2 * H], f32r, tag="ewzr")
            nc.sync.dma_start(EWzr_sb[96:97, :], bzr_d[:, :])
            EWh_dram = dpool.tile([V, H], f32)
            for nb in range(3):
                ps = ppool.tile([V, 512], f32, tag="rpz")
                for c in range(EC):
                    nc.tensor.matmul(
                        ps[:],
                        lhsT=embT_sb[:, c * V:(c + 1) * V],
                        rhs=W_sb[:, c * G + nb * 512:c * G + (nb + 1) * 512],
                        start=(c == 0), stop=(c == EC - 1),
                    )
                if nb < 2:
                    nc.vector.tensor_copy(EWzr_sb[:V, nb * 512:(nb + 1) * 512], ps[:])
                else:
                    ew_st = gpool.tile([V, 512], f32, tag="ewst")
                    nc.vector.tensor_add(ew_st[:], ps[:], bh_sb[:])
                    nc.sync.dma_start(EWh_dram[:, :], ew_st[:])

            # ---- recurrence ----
            # h is kept as 4 column-chunk tiles [128,128] so the gate tail,
            # transposes and next-step matmuls pipeline chunk by chunk.
            zero_f = cpool.tile([P, BL], f32, tag="zerof")
            nc.vector.memset(zero_f[:], 0.0)
            h_cur = []
            hT_cur = []
            for kc in range(KC):
                hj = hpool.tile([P, P], f32, tag="h")
                nc.vector.tensor_copy(hj[:], zero_f[:])
                h_cur.append(hj)
                ht = htpool.tile([P, BL], f32r)
                nc.vector.tensor_copy(ht[:], zero_f[:])
                hT_cur.append(ht)

            for t in range(T):
                # x-projection for the h gate: gather rows of EW_h
                xph = xpool.tile([BL, H], f32)
                nc.gpsimd.indirect_dma_start(
                    out=xph[:], out_offset=None, in_=EWh_dram[:, :],
                    in_offset=bass.IndirectOffsetOnAxis(ap=ids_sb[:, t:t + 1], axis=0),
                )

                # one-hot chunk [v, b] for this step: row v=id_b gets 1.0;
                # row 96 = 1.0 everywhere (bias row of EWzr)
                idsb = ohpool.tile([P, BL], f32, tag="idsb")
                nc.sync.dma_start(idsb[:], idsT_d[t:t + 1, :].to_broadcast([P, BL]))
                ohT = ohpool.tile([P, BL], f32r, tag="ohT")
                nc.gpsimd.tensor_scalar(
                    out=ohT[:], in0=idsb[:], scalar1=iota_p[:, :1], scalar2=None,
                    op0=OP.is_equal,
                )
                nc.gpsimd.tensor_copy(ohT[V:V + 1, :], ones_f[:, :])

                ps_z = ppool.tile([BL, 512], f32, tag="rpz")
                ps_r = ppool.tile([BL, 512], f32, tag="rpr")
                ps_h = ppool.tile([BL, 512], f32, tag="rph")
                for nb, ps in enumerate((ps_z, ps_r, ps_h)):
                    for kc in range(KC):
                        nc.tensor.matmul(
                            ps[:], lhsT=hT_cur[kc][:],
                            rhs=U_sb[:, kc * G + nb * 512:kc * G + (nb + 1) * 512],
                            start=(kc == 0), stop=False,
                        )
                    if nb == 0:
                        nc.tensor.matmul(
                            ps[:], lhsT=ohT[:V + 1, :], rhs=EWzr_sb[:V + 1, 0:512],
                            start=False, stop=True,
                        )
                        s = gpool.tile([BL, 512], f32, tag="s")
                        nc.scalar.activation(s[:], ps[:], AF.Sigmoid, scale=-1.0)
                    elif nb == 1:
                        nc.tensor.matmul(
                            ps[:], lhsT=ohT[:V + 1, :], rhs=EWzr_sb[:V + 1, 512:1024],
                            start=False, stop=True,
                        )
                        r = gpool.tile([BL, 512], f32, tag="r")
                        nc.scalar.activation(r[:], ps[:], AF.Sigmoid)
                    else:
                        nc.tensor.matmul(
                            ps[:], lhsT=ones_sb[:, :], rhs=brh_sb[:, :],
                            start=False, stop=True,
                        )

                # gate tail, one 128-column chunk at a time
                last = t == T - 1
                if not last:
                    pst = ppool.tile([P, 512], f32, tag="pst")
                h_new = []
                hT_new = []
                for j in range(KC):
                    sl = slice(j * P, (j + 1) * P)
                    t3 = gpool.tile([BL, P], f32, tag="t3")
                    nc.vector.tensor_mul(t3[:], r[:, sl], ps_h[:, sl])
                    ah = gpool.tile([BL, P], f32, tag="ah")
                    nc.vector.tensor_add(ah[:], t3[:], xph[:, sl])
                    s2 = gpool.tile([BL, P], f32, tag="s2")
                    nc.scalar.activation(s2[:], ah[:], AF.Sigmoid, scale=2.0)
                    c = gpool.tile([BL, P], f32, tag="c")
                    nc.vector.scalar_tensor_tensor(
                        out=c[:], in0=s2[:], scalar=2.0, in1=h_cur[j][:],
                        op0=OP.mult, op1=OP.subtract,
                    )  # 2*sigmoid(2*ah) - h = tanh(ah) - h + 1
                    e1 = gpool.tile([BL, P], f32, tag="e1")
                    nc.vector.scalar_tensor_tensor(
                        out=e1[:], in0=c[:], scalar=-1.0, in1=s[:, sl],
                        op0=OP.add, op1=OP.mult,
                    )  # (hh - h) * (1-z)
                    hj = hpool.tile([BL, P], f32, tag="h")
                    nc.vector.scalar_tensor_tensor(
                        out=hj[:], in0=e1[:], scalar=maskf[:, t:t + 1], in1=h_cur[j][:],
                        op0=OP.mult, op1=OP.add,
                    )  # h + m*(1-z)*(hh-h)
                    h_new.append(hj)
                    nc.sync.dma_start(seq_d[:, t, sl], hj[:])
                    if not last:
                        nc.tensor.transpose(pst[:, sl], hj[:], ident[:])
                        ht = htpool.tile([P, BL], f32r)
                        nc.scalar.copy(ht[:], pst[:, sl])
                        hT_new.append(ht)
                if last:
                    for j in range(KC):
                        nc.sync.dma_start(hid_d[:, j * P:(j + 1) * P], h_new[j][:])
                    break
                h_cur, hT_cur = h_new, hT_new

    nc.finalize()
    return nc


def _get_nc():
    if "nc" not in _CACHE:
        _CACHE["nc"] = _build_nc()
    return _CACHE["nc"]


def _pack_rows(M, chunk):
    """[R, C] -> [chunk, (R//chunk)*C] with row-chunks side by side."""
    R, C = M.shape
    return np.concatenate([M[i * chunk:(i + 1) * chunk] for i in range(R // chunk)],
                          axis=1)


def _in_maps(src_ids, emb, W_f, U_f, b_f, W_b, U_b, b_b):
    f = np.float32
    maps = []
    for direction, (Wd, Ud, bd) in enumerate(((W_f, U_f, b_f), (W_b, U_b, b_b))):
        Wd = np.asarray(Wd, f)
        Ud = np.asarray(Ud, f)
        bd = np.asarray(bd, f)
        embT = np.asarray(emb, f).T  # [E, V]
        bias_zr = (bd[0, :2 * H] + bd[1, :2 * H])[None, :]
        shared = {
            "embT_p": np.ascontiguousarray(_pack_rows(embT, P)),
            "W_p": np.ascontiguousarray(_pack_rows(Wd, P)),
            "U_p": np.ascontiguousarray(_pack_rows(Ud, P)),
            "bias_zr": np.ascontiguousarray(bias_zr),
            "bias_h_rep": np.ascontiguousarray(
                np.tile(bd[0, 2 * H:][None, :], (V, 1))),
            "brh_row": np.ascontiguousarray(bd[1, 2 * H:][None, :]),
        }
        for q in range(4):
            ids = np.ascontiguousarray(src_ids[q * BL:(q + 1) * BL], dtype=np.int32)
            if direction == 1:
                ids = np.ascontiguousarray(ids[:, ::-1])
            maps.append({
                "ids": ids,
                "idsT": np.ascontiguousarray(ids.T.astype(np.float32)),
                **shared,
            })
    return maps


def kernel(src_ids, emb, W_f, U_f, b_f, W_b, U_b, b_b):
    from concourse.bass_utils import run_bass_kernel_spmd

    nc = _get_nc()
    maps = _in_maps(src_ids, emb, W_f, U_f, b_f, W_b, U_b, b_b)
    res = run_bass_kernel_spmd(nc, maps, core_ids=list(range(8))).results

    seq_f = np.concatenate([res[q]["seq"] for q in range(4)], axis=0)
    seq_b = np.concatenate([res[4 + q]["seq"] for q in range(4)], axis=0)[:, ::-1, :]
    sequences = np.concatenate([seq_f, seq_b], axis=-1)
    hid_f = np.concatenate([res[q]["hid"] for q in range(4)], axis=0)
    hid_b = np.concatenate([res[4 + q]["hid"] for q in range(4)], axis=0)
    enc_hid = np.concatenate([hid_f, hid_b], axis=-1)
    mask = np.asarray(src_ids) != 0
    return sequences, enc_hid, mask
